# revision 1
# baseline (speedup 1.0000x reference)
"""Causal self-attention kernel for 8 TRN2 NeuronCores.

Sharding: 8 cores = 4 batches x 2 head-groups (8 heads / 512 channels each).
Each core computes q/k/v projections for its head half, causal attention for
its 8 heads, and a partial c_proj contracted over its 512 channels. The host
sums the two partials per batch and adds the c_proj bias.

All matmuls run in bf16 with fp32 PSUM accumulation. Host pre-transposes
x and the weight matrices so the device only ever does natural-layout DMAs.

Device layouts (per core):
  xt_sb [128, 8, T]        x^T tiles: [cin%128, cin//128, t]
  qt/kt_sb [128, 4, T]     Q^T/K^T: [d%128, head-pair, t] (head 2p: rows 0-63)
  v1_sb [128, T//128, 65, 8]  V interleaved [t%128, t//128, j, h]; col j<64 is
                           V_h[d=j] (host permutes Wv cols to dd*8+h), j=64
                           is the ones column used to accumulate the softmax
                           denominator inside the P@V matmul (M=65).

Scores are computed transposed (S^T[k, q]) so softmax'd probabilities feed
P@V directly as the moving operand; exp runs on ScalarE from PSUM with the
1/sqrt(hd) scale folded in; diagonal blocks are masked post-exp and trimmed
to their valid q-range. The attention inner loop is ScalarE-paced, so
projection / c_proj matmul groups are interleaved into it at k-tile
granularity to keep TensorE busy, and P@V is emitted one k-tile behind the
exp that feeds it.
"""

import numpy as np
import ml_dtypes
from contextlib import ExitStack

import concourse.bass as bass
import concourse.tile as tile
from concourse import bacc, mybir
from concourse.bass_utils import run_bass_kernel_spmd

BF16 = mybir.dt.bfloat16
F32 = mybir.dt.float32

N_EMBD = 1024
N_HEAD = 16
B = 4
T_FULL = 2048
HD = 64           # head dim
HPC = 8           # heads per core
CH = HPC * HD     # channels per core = 512
N_CORES = 8
SCALE = 1.0 / 8.0  # 1/sqrt(HD)

P = 128           # partitions
QC = 512          # q-chunk (matmul free dim)


def build_nc(T=T_FULL, pack_qk=True):
    """Build the per-core Bass module (same program on every core)."""
    n_tt = T // P          # 128-row tiles along T
    n_qc = T // QC         # 512-wide chunks along T
    n_ci = N_EMBD // P     # contraction tiles over the full embed dim
    n_dt = CH // P         # d-tiles of this core's 512 channels (= head pairs)
    n_oc = N_EMBD // QC

    nc = bacc.Bacc("TRN2", target_bir_lowering=False, debug=False)

    xt0 = nc.dram_tensor("xt0", [N_EMBD, QC], BF16, kind="ExternalInput").ap()
    xtr = (nc.dram_tensor("xtr", [N_EMBD, T - QC], BF16,
                          kind="ExternalInput").ap() if T > QC else None)
    wqkv = nc.dram_tensor("wqkv", [N_EMBD, 3 * CH], BF16,
                          kind="ExternalInput").ap()
    wc = nc.dram_tensor("wc", [CH, N_EMBD], BF16, kind="ExternalInput").ap()
    bq = nc.dram_tensor("bq", [P, n_dt], F32, kind="ExternalInput").ap()
    bk = nc.dram_tensor("bk", [P, n_dt], F32, kind="ExternalInput").ap()
    vb1 = nc.dram_tensor("vb1", [P, CH + HPC], F32, kind="ExternalInput").ap()
    masks = nc.dram_tensor("masks", [P, 4 * QC], BF16, kind="ExternalInput").ap()
    out = nc.dram_tensor("out", [T, N_EMBD], F32, kind="ExternalOutput").ap()

    with tile.TileContext(nc) as tc, ExitStack() as ctx:
        singles = ctx.enter_context(tc.tile_pool(name="singles", bufs=1))
        mm_ps = ctx.enter_context(tc.tile_pool(name="mm_ps", bufs=2, space="PSUM"))
        qk_ps_pool = ctx.enter_context(tc.tile_pool(name="qk_ps", bufs=3, space="PSUM"))
        av_ps_pool = ctx.enter_context(tc.tile_pool(name="av_ps", bufs=3, space="PSUM"))
        pt_pool = ctx.enter_context(tc.tile_pool(name="pt", bufs=12))
        small = ctx.enter_context(tc.tile_pool(name="small", bufs=4))
        dram = ctx.enter_context(tc.tile_pool(name="dram", bufs=4, space="DRAM"))
        ost = ctx.enter_context(tc.tile_pool(name="ost", bufs=4))

        # ---- resident tensors (split DMAs so compute starts early) ----
        xt_sb = singles.tile([P, n_ci, T], BF16)
        wqkv_sb = singles.tile([P, n_ci, 3 * CH], BF16)
        wq_sb = wqkv_sb[:, :, 0:CH]
        wk_sb = wqkv_sb[:, :, CH:2 * CH]
        wv_sb = wqkv_sb[:, :, 2 * CH:3 * CH]
        for ci in range(n_ci):
            rows = slice(ci * P, (ci + 1) * P)
            nc.sync.dma_start(xt_sb[:, ci, 0:QC], xt0[rows, :])
            nc.sync.dma_start(wqkv_sb[:, ci, :], wqkv[rows, :])
        bq_sb = singles.tile([P, n_dt], F32)
        nc.sync.dma_start(bq_sb, bq)
        bk_sb = singles.tile([P, n_dt], F32)
        nc.sync.dma_start(bk_sb, bk)
        vb1_sb = singles.tile([P, CH + HPC], F32)
        nc.sync.dma_start(vb1_sb, vb1)
        mask_sb = singles.tile([P, 4, QC], BF16)
        nc.sync.dma_start(mask_sb, masks.rearrange("p (r q) -> p r q", r=4))
        if xtr is not None:
            for ci in range(n_ci):
                nc.sync.dma_start(xt_sb[:, ci, QC:], xtr[ci * P:(ci + 1) * P, :])
        wc_sb = singles.tile([P, n_dt, N_EMBD], BF16)
        for pr in range(n_dt):
            nc.sync.dma_start(wc_sb[:, pr, :], wc[pr * P:(pr + 1) * P, :])

        qt_sb = singles.tile([P, n_dt, T], BF16)   # [d%128, head-pair, t]
        kt_sb = singles.tile([P, n_dt, T], BF16)
        v1_sb = singles.tile([P, n_tt, HD + 1, HPC], BF16)
        yt_sb = singles.tile([P, n_dt, T], BF16)   # attention out (normalized)

        # preload the ScalarE exp table set during the input-DMA window so
        # the first real exp doesn't pay the ~2.7us ACT_TABLE_LOAD stall
        warm1 = small.tile([1, 1], F32, tag="warm1")
        nc.vector.memset(warm1, 0.0)
        nc.scalar.activation(warm1, warm1, mybir.ActivationFunctionType.Exp)

        # ones column of v1 (written once, before any V tile is consumed)
        nc.vector.tensor_copy(
            out=v1_sb[:, :, HD, :],
            in_=vb1_sb[:, None, CH:].to_broadcast((P, n_tt, HPC)),
        )

        # ---- work units ----
        def emit_v(tt, pool=None, tag="mm"):
            ps = (pool or mm_ps).tile([P, CH], F32, tag=tag, name=f"v_{tt}")
            for ci in range(n_ci):
                nc.tensor.matmul(
                    ps,
                    lhsT=xt_sb[:, ci, tt * P:(tt + 1) * P],
                    rhs=wv_sb[:, ci, :],
                    start=(ci == 0), stop=(ci == n_ci - 1),
                )
            nc.vector.tensor_add(
                out=v1_sb[:, tt, 0:HD, :],
                in0=ps.rearrange("p (j h) -> p j h", h=HPC),
                in1=vb1_sb[:, 0:CH].rearrange("p (j h) -> p j h", h=HPC),
            )

        def emit_proj(w_sb, b_sb, o_sb, pr, tcn, pool=None, tag="mm"):
            ps = (pool or mm_ps).tile([P, QC], F32, tag=tag,
                                      name=f"pj_{pr}_{tcn}")
            for ci in range(n_ci):
                nc.tensor.matmul(
                    ps,
                    lhsT=w_sb[:, ci, pr * P:(pr + 1) * P],
                    rhs=xt_sb[:, ci, tcn * QC:(tcn + 1) * QC],
                    start=(ci == 0), stop=(ci == n_ci - 1),
                )
            nc.vector.tensor_add(
                out=o_sb[:, pr, tcn * QC:(tcn + 1) * QC],
                in0=ps,
                in1=b_sb[:, pr, None].to_broadcast((P, QC)),
            )

        def emit_cproj(tt, oc, on_act=False):
            ps = mm_ps.tile([P, QC], F32, tag="mm")
            for pr in range(n_dt):
                nc.tensor.matmul(
                    ps,
                    lhsT=yt_sb[:, pr, tt * P:(tt + 1) * P],
                    rhs=wc_sb[:, pr, oc * QC:(oc + 1) * QC],
                    start=(pr == 0), stop=(pr == n_dt - 1),
                )
            st = ost.tile([P, QC], F32, tag="ost")
            if on_act:  # ScalarE is idle once attention's exp stream drains
                nc.scalar.activation(st, ps, mybir.ActivationFunctionType.Copy)
            else:
                nc.vector.tensor_copy(out=st, in_=ps)
            nc.sync.dma_start(
                out=out[tt * P:(tt + 1) * P, oc * QC:(oc + 1) * QC], in_=st,
            )

        # ---- fused pipeline over q-chunks ----
        # prologue: V for chunk 0 and Q/K projections for (pr=0, chunk 0).
        # Attention PSUM pools are idle during the input-DMA window, so route
        # prologue groups through them for more load-tracking concurrency.
        for tt in range(QC // P):
            emit_v(tt, pool=qk_ps_pool if tt % 2 else None,
                   tag="qk" if tt % 2 else "mm")
        emit_proj(wq_sb, bq_sb, qt_sb, 0, 0, pool=qk_ps_pool, tag="qk")
        emit_proj(wk_sb, bk_sb, kt_sb, 0, 0)

        order = list(range(n_qc))
        projected = {0}
        for s, qc in enumerate(order):
            q0 = qc * QC
            nkt = (q0 + QC) // P  # causal: k-tiles 0..nkt-1
            last = s == n_qc - 1

            # filler units: projections for the next chunk in process order;
            # all deferrable c_proj is pushed into the final stage, whose
            # attention leaves TensorE the most idle.
            fillers = []
            if s + 1 < n_qc:
                # attention(next) needs Q^T for its own chunk and K^T/V for
                # every chunk up to it: emit all not-yet-projected chunks
                for c in range(order[s + 1] + 1):
                    if c not in projected:
                        projected.add(c)
                        for tt in range(c * (QC // P), (c + 1) * (QC // P)):
                            fillers.append(("v", tt))
                        for pr in range(n_dt):
                            fillers.append(("q", pr, c))
                            fillers.append(("k", pr, c))
            if last:
                for done in order[:-1]:
                    for tt in range(done * (QC // P), (done + 1) * (QC // P)):
                        for oc in range(n_oc):
                            fillers.append(("c", tt, oc))
            n_slots = n_dt * nkt
            per_slot = len(fillers) / n_slots
            if last:
                per_slot *= 0.6  # hold filler back for the drain at chunk end
            facc = 0.0
            fi = 0

            for pr in range(n_dt):
                if s == 0 and pr >= 1:
                    emit_proj(wq_sb, bq_sb, qt_sb, pr, 0,
                              pool=qk_ps_pool, tag="qk")
                    emit_proj(wk_sb, bk_sb, kt_sb, pr, 0)
                av_A = av_ps_pool.tile([HD + 1, QC], F32, tag="av")
                av_B = av_ps_pool.tile([HD + 1, QC], F32, tag="av")
                pend = []  # delayed P@V queue: (kt, relq, pts)
                
                for kt in range(nkt):
                    k0 = kt * P
                    rel = kt - (q0 // P)  # >=0 on diagonal block tiles
                    relq = rel * P if rel > 0 else 0
                    pts = []
                    for hi, prt in enumerate((slice(0, HD), slice(HD, P))):
                        qk = qk_ps_pool.tile([P, QC], F32, tag="qk")
                        nc.tensor.matmul(
                            qk[:, relq:],
                            lhsT=kt_sb[prt, pr, k0:k0 + P],
                            rhs=qt_sb[prt, pr, q0 + relq:q0 + QC],
                            start=True, stop=True,
                            tile_position=(prt.start, 0) if pack_qk else None,
                        )
                        pt = pt_pool.tile([P, QC], BF16, tag="pt")
                        nc.scalar.activation(
                            pt[:, relq:], qk[:, relq:],
                            mybir.ActivationFunctionType.Exp, scale=SCALE,
                        )
                        if rel >= 0:
                            nc.vector.tensor_mul(
                                pt[:, relq:], pt[:, relq:],
                                mask_sb[:, rel, relq:],
                            )
                        pts.append(pt)
                    pend.append((kt, relq, pts))
                    if len(pend) > 5:
                        pkt, prelq, ppts = pend.pop(0)
                        for hi, av in enumerate((av_A, av_B)):
                            nc.tensor.matmul(
                                av[:, prelq:],
                                lhsT=v1_sb[:, pkt, :, 2 * pr + hi],
                                rhs=ppts[hi][:, prelq:],
                                start=(pkt == 0), stop=(pkt == nkt - 1),
                            )
                    # interleave filler work to keep TensorE fed
                    facc += per_slot
                    while facc >= 1.0 and fi < len(fillers):
                        f = fillers[fi]
                        fi += 1
                        facc -= 1.0
                        if f[0] == "v":
                            emit_v(f[1])
                        elif f[0] == "q":
                            emit_proj(wq_sb, bq_sb, qt_sb, f[1], f[2])
                        elif f[0] == "k":
                            emit_proj(wk_sb, bk_sb, kt_sb, f[1], f[2])
                        else:
                            emit_cproj(f[1], f[2])
                # drain delayed P@V queue
                while pend:
                    pkt, prelq, ppts = pend.pop(0)
                    for hi, av in enumerate((av_A, av_B)):
                        nc.tensor.matmul(
                            av[:, prelq:],
                            lhsT=v1_sb[:, pkt, :, 2 * pr + hi],
                            rhs=ppts[hi][:, prelq:],
                            start=(pkt == 0), stop=(pkt == nkt - 1),
                        )
                # normalize: y[d, q] = av[d, q] / av[HD, q]; one DRAM bounce
                # broadcasts both heads' fp32 reciprocal rows across partitions
                r_sb = small.tile([HD + 1, 2, QC], F32, tag="recip")
                nc.vector.reciprocal(out=r_sb[HD:HD + 1, 0, :],
                                     in_=av_A[HD:HD + 1, :])
                nc.vector.reciprocal(out=r_sb[HD:HD + 1, 1, :],
                                     in_=av_B[HD:HD + 1, :])
                dscr = dram.tile([1, 2 * QC], F32, tag="dbounce")
                nc.sync.dma_start(
                    out=dscr, in_=r_sb[HD:HD + 1].rearrange("p a q -> p (a q)"))
                den_sb = small.tile([HD, 2, QC], F32, tag="den")
                nc.sync.dma_start(
                    out=den_sb,
                    in_=bass.AP(tensor=dscr.tensor, offset=dscr.offset,
                                ap=[[0, HD], [QC, 2], [1, QC]]),
                )
                tmp = small.tile([HD, QC], BF16, tag="ytmp")
                nc.vector.tensor_mul(out=tmp, in0=av_B[0:HD, :],
                                     in1=den_sb[:, 1, :])
                nc.sync.dma_start(out=yt_sb[HD:P, pr, q0:q0 + QC], in_=tmp)
                nc.vector.tensor_mul(
                    out=yt_sb[0:HD, pr, q0:q0 + QC],
                    in0=av_A[0:HD, :], in1=den_sb[:, 0, :],
                )

                if last:
                    # drain held-back filler while the next pair's exps queue
                    for _ in range(1):
                        if fi < len(fillers):
                            f = fillers[fi]
                            fi += 1
                            if f[0] == "c":
                                emit_cproj(f[1], f[2])

            # any leftover fillers for this stage
            while fi < len(fillers):
                f = fillers[fi]
                fi += 1
                if f[0] == "v":
                    emit_v(f[1])
                elif f[0] == "q":
                    emit_proj(wq_sb, bq_sb, qt_sb, f[1], f[2])
                elif f[0] == "k":
                    emit_proj(wk_sb, bk_sb, kt_sb, f[1], f[2])
                else:
                    emit_cproj(f[1], f[2])

        # keep TensorE warm through the last normalize's latency chain so the
        # epilogue c_proj runs at full clock (results are never read)
        for i in range(8):
            wps = qk_ps_pool.tile([P, QC], F32, tag="qk", name=f"warm_{i}")
            nc.tensor.matmul(
                wps, lhsT=kt_sb[0:HD, 0, 0:P], rhs=qt_sb[0:HD, 0, 0:QC],
                start=True, stop=True,
            )

        # epilogue: c_proj for the final-stage chunk (copies on idle ScalarE)
        for tt in range(order[-1] * (QC // P), (order[-1] + 1) * (QC // P)):
            for oc in range(n_oc):
                emit_cproj(tt, oc, on_act=True)

    nc.compile()
    return nc


def make_in_maps(x, Wq, bq, Wk, bk, Wv, bv, T=T_FULL):
    """Host-side sharding + layout prep. Returns per-core input dicts."""
    bf = ml_dtypes.bfloat16
    x = np.asarray(x, dtype=np.float32)
    n_dt = CH // P

    # causal masks for the 4 diagonal-relative offsets
    k_idx = np.arange(P)[:, None]
    q_idx = np.arange(QC)[None, :]
    masks = np.concatenate(
        [(r * P + k_idx <= q_idx) for r in range(4)], axis=1
    ).astype(bf)  # [128, 4*512] packed along the free dim

    # head-interleave permutation for Wv columns: new col j*HPC+h = old h*HD+j
    j = np.arange(HD)[:, None]
    h = np.arange(HPC)[None, :]
    perm = (h * HD + j).reshape(-1)  # new[j*HPC+h] <- old[h*HD+j]

    wqT = np.ascontiguousarray(Wq.T).astype(bf)  # [cin, dout]
    wkT = np.ascontiguousarray(Wk.T).astype(bf)
    wvT = np.ascontiguousarray(Wv.T).astype(bf)

    in_maps = []
    for core in range(N_CORES):
        b = core // 2
        hh = core % 2
        cs = slice(hh * CH, (hh + 1) * CH)
        xtb = np.ascontiguousarray(x[b, :T].T).astype(bf)  # [N_EMBD, T]
        xt0 = np.ascontiguousarray(xtb[:, :QC])
        xtr = np.ascontiguousarray(xtb[:, QC:])

        bq_arr = np.asarray(bq[cs], np.float32).reshape(n_dt, P).T.copy()
        bk_arr = np.asarray(bk[cs], np.float32).reshape(n_dt, P).T.copy()
        bv_half = np.asarray(bv[cs], np.float32)
        vb1 = np.concatenate([bv_half[perm], np.ones(HPC, np.float32)])
        vb1 = np.broadcast_to(vb1, (P, CH + HPC)).copy()

        im = {
            "xt0": xt0,
            "wqkv": np.ascontiguousarray(np.concatenate(
                [wqT[:, cs], wkT[:, cs], wvT[:, cs][:, perm]], axis=1)),
            "wc": None,  # filled by caller (needs Wc)
            "bq": bq_arr,
            "bk": bk_arr,
            "vb1": vb1,
            "masks": masks,
        }
        if T > QC:
            im["xtr"] = xtr
        in_maps.append(im)
    return in_maps


_NC_CACHE = {}


def kernel(x, Wq, bq, Wk, bk, Wv, bv, Wc, bc):
    x = np.asarray(x, dtype=np.float32)
    T = x.shape[1]
    key = T
    if key not in _NC_CACHE:
        _NC_CACHE[key] = build_nc(T=T)
    nc = _NC_CACHE[key]

    in_maps = make_in_maps(x, Wq, bq, Wk, bk, Wv, bv, T=T)
    wcT = np.ascontiguousarray(np.asarray(Wc, np.float32).T).astype(
        ml_dtypes.bfloat16)  # [cin, cout]
    for core in range(N_CORES):
        hh = core % 2
        in_maps[core]["wc"] = np.ascontiguousarray(wcT[hh * CH:(hh + 1) * CH, :])

    res = run_bass_kernel_spmd(nc, in_maps, core_ids=list(range(N_CORES)))

    bc = np.asarray(bc, np.float32)
    out = np.empty((B, T, N_EMBD), np.float32)
    for b in range(B):
        out[b] = res.results[2 * b]["out"] + res.results[2 * b + 1]["out"] + bc
    return out



# revision 5
# speedup vs baseline: 1.2139x; 1.2139x over previous
"""Causal self-attention kernel for 8 TRN2 NeuronCores.

Sharding: 8 cores = 4 batches x 2 head-groups (8 heads / 512 channels each).
Each core computes q/k/v projections for its head half, causal attention for
its 8 heads, and a partial c_proj contracted over its 512 channels. The host
sums the two partials per batch and adds the c_proj bias.

v2 dataflow (per core):
  - q/k are written as fp8e4 with a zero second DoubleRow lane
    ([128d, pr, 2, T]); score matmuls run in MatmulPerfMode.DoubleRow
    (half cost per streamed column), contracting [64 part x 2 lanes]
    where lane 1 is zero.  Scores land transposed (S^T[k, q]) in fp32
    PSUM quadrants [128, 2hi, 2half, 256].
  - exp runs once per (pr, kt) over the fused [2, 2, 256] quadrants.
  - P@V is probs-stationary: lhsT = P^T[128k, 128q] block, moving
    operand = V||ones [128k, 65] -> accumulates y[q, d] + denominator
    per q-tile in PSUM.  Normalization is a per-partition reciprocal
    multiply, then a DMA-engine transpose moves y[q, d] -> yt[d, q]
    for c_proj.
  - a deadline-aware pacer interleaves projection / c_proj groups into
    the attention stream so TensorE stays fed while ScalarE grinds exp.
"""

import numpy as np
import ml_dtypes
from collections import deque
from contextlib import ExitStack

import concourse.bass as bass
import concourse.tile as tile
from concourse import bacc, mybir
from concourse.bass_utils import run_bass_kernel_spmd

BF16 = mybir.dt.bfloat16
F32 = mybir.dt.float32
FP8 = mybir.dt.float8e4

N_EMBD = 1024
N_HEAD = 16
B = 4
T_FULL = 2048
HD = 64           # head dim
HPC = 8           # heads per core
CH = HPC * HD     # channels per core = 512
N_CORES = 8
SCALE = 1.0 / 8.0  # 1/sqrt(HD)

P = 128           # partitions
QC = 512          # q-chunk
HC = 256          # half-chunk (DoubleRow moving-dim limit)

# pacer cost constants (ns)
PE_NS = 1.0 / 2.4
ACT_NS = 1.0 / 1.2
DVE_NS = 1.0 / 0.96
EXP_OVH = 370.0
DVE_OVH = 250.0
LEAD_NS = 2500.0
LAG = 2           # P@V trails exp by this many k-tiles


def build_nc(T=T_FULL):
    n_tt = T // P
    n_qc = T // QC
    n_ci = N_EMBD // P
    n_dt = CH // P
    n_oc = N_EMBD // QC

    nc = bacc.Bacc("TRN2", target_bir_lowering=False, debug=False)

    xt0 = nc.dram_tensor("xt0", [N_EMBD, QC], BF16, kind="ExternalInput").ap()
    xtr = (nc.dram_tensor("xtr", [N_EMBD, T - QC], BF16,
                          kind="ExternalInput").ap() if T > QC else None)
    wqkv = nc.dram_tensor("wqkv", [N_EMBD, 3 * CH], BF16,
                          kind="ExternalInput").ap()
    wc = nc.dram_tensor("wc", [CH, N_EMBD], BF16, kind="ExternalInput").ap()
    bq = nc.dram_tensor("bq", [P, n_dt], F32, kind="ExternalInput").ap()
    bk = nc.dram_tensor("bk", [P, n_dt], F32, kind="ExternalInput").ap()
    vb1 = nc.dram_tensor("vb1", [P, CH + HPC], F32, kind="ExternalInput").ap()
    masks = nc.dram_tensor("masks", [P, 4 * QC], BF16, kind="ExternalInput").ap()
    out = nc.dram_tensor("out", [T, N_EMBD], F32, kind="ExternalOutput").ap()

    with tile.TileContext(nc) as tc, ExitStack() as ctx:
        singles = ctx.enter_context(tc.tile_pool(name="singles", bufs=1))
        mm_ps = ctx.enter_context(tc.tile_pool(name="mm_ps", bufs=2, space="PSUM"))
        qk_ps = ctx.enter_context(tc.tile_pool(name="qk_ps", bufs=2, space="PSUM"))
        av_ps = ctx.enter_context(tc.tile_pool(name="av_ps", bufs=2, space="PSUM"))
        pt_pool = ctx.enter_context(tc.tile_pool(name="pt", bufs=6))
        y_pool = ctx.enter_context(tc.tile_pool(name="yp", bufs=6))
        r_pool = ctx.enter_context(tc.tile_pool(name="rp", bufs=6))
        ost = ctx.enter_context(tc.tile_pool(name="ost", bufs=3))

        # ---- resident tensors ----
        xt_sb = singles.tile([P, n_ci, T], BF16)
        wqkv_sb = singles.tile([P, n_ci, 3 * CH], BF16)
        wq_sb = wqkv_sb[:, :, 0:CH]
        wk_sb = wqkv_sb[:, :, CH:2 * CH]
        wv_sb = wqkv_sb[:, :, 2 * CH:3 * CH]
        wc_sb = singles.tile([P, n_dt, N_EMBD], BF16)
        bq_sb = singles.tile([P, n_dt], F32)
        bk_sb = singles.tile([P, n_dt], F32)
        vb1_sb = singles.tile([P, CH + HPC], F32)
        mask_sb = singles.tile([P, 4, QC], BF16)
        q8 = singles.tile([P, n_dt, 2, T], FP8)   # [d, pr, lane, t]; lane1 = 0
        k8 = singles.tile([P, n_dt, 2, T], FP8)
        v1_sb = singles.tile([P, n_tt, HD + 1, HPC], BF16)
        yt_sb = singles.tile([P, n_dt, T], BF16)

        # ---- input DMAs: first ci pair, then small tensors, then the rest
        nc.sync.dma_start(xt_sb[:, 0, 0:QC], xt0[0:P, :])
        nc.sync.dma_start(wqkv_sb[:, 0, :], wqkv[0:P, :])
        nc.sync.dma_start(bq_sb, bq)
        nc.sync.dma_start(bk_sb, bk)
        nc.sync.dma_start(vb1_sb, vb1)
        nc.sync.dma_start(mask_sb, masks.rearrange("p (r q) -> p r q", r=4))
        for ci in range(1, n_ci):
            rows = slice(ci * P, (ci + 1) * P)
            nc.sync.dma_start(xt_sb[:, ci, 0:QC], xt0[rows, :])
            nc.sync.dma_start(wqkv_sb[:, ci, :], wqkv[rows, :])
        if xtr is not None:
            xtr_r = xtr.rearrange("(c p) t -> p c t", p=P)
            half = n_ci // 2
            nc.sync.dma_start(xt_sb[:, 0:half, QC:], xtr_r[:, 0:half, :])
            nc.sync.dma_start(xt_sb[:, half:, QC:], xtr_r[:, half:, :])
        nc.sync.dma_start(wc_sb, wc.rearrange("(c p) o -> p c o", p=P))

        # zero DoubleRow lanes on the idle Pool engine (chunk 0 first)
        for c in range(n_qc):
            cs = slice(c * QC, (c + 1) * QC)
            nc.gpsimd.memset(k8[:, :, 1, cs], 0.0)
            nc.gpsimd.memset(q8[:, :, 1, cs], 0.0)

        # ones column of v1
        nc.vector.tensor_copy(
            out=v1_sb[:, :, HD, :],
            in_=vb1_sb[:, None, CH:].to_broadcast((P, n_tt, HPC)),
        )

        # ---- pacer state ----
        st = {"pe": 0.0, "act": 0.0, "dve": 0.0}

        def emit_v(tt):
            ps = mm_ps.tile([P, CH], F32, tag="mm", name=f"v_{tt}")
            for ci in range(n_ci):
                nc.tensor.matmul(
                    ps,
                    lhsT=xt_sb[:, ci, tt * P:(tt + 1) * P],
                    rhs=wv_sb[:, ci, :],
                    start=(ci == 0), stop=(ci == n_ci - 1),
                )
            nc.vector.tensor_add(
                out=v1_sb[:, tt, 0:HD, :],
                in0=ps.rearrange("p (j h) -> p j h", h=HPC),
                in1=vb1_sb[:, 0:CH].rearrange("p (j h) -> p j h", h=HPC),
            )
            st["pe"] += n_ci * CH * PE_NS
            st["dve"] += CH * DVE_NS + DVE_OVH

        def emit_proj(which, pr, tcn):
            w_sb, b_sb, o8 = ((wq_sb, bq_sb, q8) if which == "q"
                              else (wk_sb, bk_sb, k8))
            ps = mm_ps.tile([P, QC], F32, tag="mm", name=f"pj{which}_{pr}_{tcn}")
            for ci in range(n_ci):
                nc.tensor.matmul(
                    ps,
                    lhsT=w_sb[:, ci, pr * P:(pr + 1) * P],
                    rhs=xt_sb[:, ci, tcn * QC:(tcn + 1) * QC],
                    start=(ci == 0), stop=(ci == n_ci - 1),
                )
            nc.vector.tensor_add(
                out=o8[:, pr, 0, tcn * QC:(tcn + 1) * QC],
                in0=ps,
                in1=b_sb[:, pr, None].to_broadcast((P, QC)),
            )
            st["pe"] += n_ci * QC * PE_NS
            st["dve"] += QC * DVE_NS + DVE_OVH

        def emit_cproj(tt, on_act=False):
            o = ost.tile([P, N_EMBD], F32, tag="ost", name=f"o_{tt}")
            for oc in range(n_oc):
                ps = mm_ps.tile([P, QC], F32, tag="mm", name=f"c_{tt}_{oc}")
                for pr2 in range(n_dt):
                    nc.tensor.matmul(
                        ps,
                        lhsT=yt_sb[:, pr2, tt * P:(tt + 1) * P],
                        rhs=wc_sb[:, pr2, oc * QC:(oc + 1) * QC],
                        start=(pr2 == 0), stop=(pr2 == n_dt - 1),
                    )
                dst = o[:, oc * QC:(oc + 1) * QC]
                if on_act:
                    nc.scalar.activation(dst, ps,
                                         mybir.ActivationFunctionType.Copy)
                    st["act"] += QC * ACT_NS + EXP_OVH
                else:
                    nc.vector.tensor_copy(out=dst, in_=ps)
                    st["dve"] += QC * DVE_NS + DVE_OVH
            nc.sync.dma_start(out=out[tt * P:(tt + 1) * P, :], in_=o)
            st["pe"] += n_oc * n_dt * QC * PE_NS

        # ---- filler machinery ----
        fillers = deque()   # items: (deadline_key or None, kind, args)
        #  deadline_key = (qc, pr, kt) before which the item must be emitted

        def emit_item(it):
            _, kind, args = it
            if kind == "v":
                emit_v(*args)
            elif kind in ("q", "k"):
                emit_proj(kind, *args)
            else:
                emit_cproj(*args)

        def force_due(pos):
            while fillers and fillers[0][0] is not None and fillers[0][0] <= pos:
                emit_item(fillers.popleft())

        def pump():
            while fillers and st["pe"] < max(st["act"], st["dve"]) + LEAD_NS:
                emit_item(fillers.popleft())

        # filler list with deadlines (sorted by construction):
        #  Q/K(pr, c0) for pr>=1 before their stage-0 segments;
        #  for chunks c>=1: Q(pr, c) before (c, pr, 0); K(pr, c) before its
        #  first diagonal use (c, pr, 4c); V(c) before (c, 0, 4c).
        pre = []
        for pr in range(1, n_dt):
            pre.append(((0, pr, 0), "q", (pr, 0)))
            pre.append(((0, pr, 0), "k", (pr, 0)))
        for c in range(1, n_qc):
            for pr in range(n_dt):
                pre.append(((c, pr, 0), "q", (pr, c)))
                pre.append(((c, pr, 4 * c), "k", (pr, c)))
            for tt in range(c * 4, c * 4 + 4):
                pre.append(((c, 0, 4 * c), "v", (tt,)))
        pre.sort(key=lambda it: it[0])
        fillers.extend(pre)

        # ---- prologue: V(c0) + Q/K(pr=0, c0), interleaved per ci ----
        qkA = qk_ps.tile([P, 2, 2, HC], F32, tag="qk", name="proA")
        qkB = qk_ps.tile([P, 2, 2, HC], F32, tag="qk", name="proB")
        mmA = mm_ps.tile([P, QC], F32, tag="mm", name="proQ")
        mmB = mm_ps.tile([P, QC], F32, tag="mm", name="proK")
        vps = [qkA.rearrange("p i h q -> p (i h q)").rearrange(
                   "p (a q) -> p a q", a=2)[:, a] for a in range(2)] + \
              [qkB.rearrange("p i h q -> p (i h q)").rearrange(
                   "p (a q) -> p a q", a=2)[:, a] for a in range(2)]
        for ci in range(n_ci):
            for tt in range(4):
                nc.tensor.matmul(
                    vps[tt],
                    lhsT=xt_sb[:, ci, tt * P:(tt + 1) * P],
                    rhs=wv_sb[:, ci, :],
                    start=(ci == 0), stop=(ci == n_ci - 1),
                )
            nc.tensor.matmul(mmA, lhsT=wq_sb[:, ci, 0:P],
                             rhs=xt_sb[:, ci, 0:QC],
                             start=(ci == 0), stop=(ci == n_ci - 1))
            nc.tensor.matmul(mmB, lhsT=wk_sb[:, ci, 0:P],
                             rhs=xt_sb[:, ci, 0:QC],
                             start=(ci == 0), stop=(ci == n_ci - 1))
        for tt in range(4):
            nc.vector.tensor_add(
                out=v1_sb[:, tt, 0:HD, :],
                in0=vps[tt].rearrange("p (j h) -> p j h", h=HPC),
                in1=vb1_sb[:, 0:CH].rearrange("p (j h) -> p j h", h=HPC),
            )
        nc.vector.tensor_add(out=q8[:, 0, 0, 0:QC], in0=mmA,
                             in1=bq_sb[:, 0, None].to_broadcast((P, QC)))
        nc.vector.tensor_add(out=k8[:, 0, 0, 0:QC], in0=mmB,
                             in1=bk_sb[:, 0, None].to_broadcast((P, QC)))
        st["pe"] += 6 * n_ci * QC * PE_NS
        st["dve"] += 6 * (QC * DVE_NS + DVE_OVH)
        st["act"] = st["dve"] = max(st["act"], st["dve"])

        # ---- main attention loop ----
        last_c = n_qc - 1
        for qc in range(n_qc):
            nkt = 4 * qc + 4
            for pr in range(n_dt):
                force_due((qc, pr, 0))
                av01 = av_ps.tile([P, 2, 2, HD + 1], F32, tag="av",
                                  name=f"av01_{qc}_{pr}")
                av23 = av_ps.tile([P, 2, 2, HD + 1], F32, tag="av",
                                  name=f"av23_{qc}_{pr}")
                avs = [av01, av01, av23, av23]
                pend = deque()

                def emit_pv(kt, ptf):
                    # PSUM zero-region semantics: one start per av bank
                    # (first write marks the whole bank pending-zero; each
                    # region's first touch then auto-overwrites), one stop on
                    # the bank's final write.
                    jmin = max(kt - 4 * qc, 0)
                    for j in range(jmin, 4):
                        gj = 4 * qc + j
                        av = avs[j]
                        for hi in range(2):
                            nc.tensor.matmul(
                                av[:, j & 1, hi, :],
                                lhsT=ptf[:, hi, j * P:(j + 1) * P],
                                rhs=v1_sb[:, kt, :, 2 * pr + hi],
                                start=(kt == 0 and hi == 0 and j in (0, 2)),
                                stop=(hi == 1 and kt == gj and j in (1, 3)),
                            )
                        st["pe"] += 2 * (HD + 1) * PE_NS
                        if hi == 1 and kt == gj and j in (1, 3):
                            # bank closed: normalize both q-tiles of the pair
                            # (reads are only legal once the group stopped),
                            # then DMA-transpose each into yt[d, q]
                            r = r_pool.tile([P, 2, 2], F32, tag="r",
                                            name=f"r_{qc}_{pr}_{j}")
                            nc.vector.reciprocal(out=r, in_=av[:, :, :, HD])
                            y = y_pool.tile([P, 2, 2, HD], BF16, tag="y",
                                            name=f"y_{qc}_{pr}_{j}")
                            nc.vector.tensor_mul(
                                out=y, in0=av[:, :, :, 0:HD],
                                in1=r[:, :, :, None].to_broadcast((P, 2, 2, HD)),
                            )
                            st["dve"] += (4 + 4 * HD) * DVE_NS + 4 * DVE_OVH
                            for jl in (j - 1, j):
                                gl = 4 * qc + jl
                                nc.sync.dma_start_transpose(
                                    yt_sb[:, pr, gl * P:(gl + 1) * P],
                                    y[:, jl & 1].rearrange("p h d -> p (h d)"),
                                )
                                if pr == n_dt - 1:
                                    fillers.append((None, "c", (gl,)))

                for kt in range(nkt):
                    force_due((qc, pr, kt))
                    rel = kt - 4 * qc
                    relq = max(rel, 0) * P
                    qk = qk_ps.tile([P, 2, 2, HC], F32, tag="qk",
                                    name=f"qk_{qc}_{pr}_{kt}")
                    for hi in range(2):
                        dd = slice(64 * hi, 64 * hi + 64)
                        hs = [h for h in range(2) if relq - h * HC < HC]
                        for h in hs:
                            s0 = max(relq - h * HC, 0)
                            nc.tensor.matmul(
                                qk[:, hi, h, s0:],
                                lhsT=k8[dd, pr, :, kt * P:(kt + 1) * P],
                                rhs=q8[dd, pr, :,
                                       qc * QC + h * HC + s0:
                                       qc * QC + (h + 1) * HC],
                                start=(h == hs[0]), stop=(h == hs[-1]),
                                perf_mode=mybir.MatmulPerfMode.DoubleRow,
                            )
                            st["pe"] += (HC - s0) * 0.5 * PE_NS
                    pt = pt_pool.tile([P, 2, 2, HC], BF16, tag="pt",
                                      name=f"pt_{qc}_{pr}_{kt}")
                    qkf = qk.rearrange("p i h q -> p i (h q)")
                    ptf = pt.rearrange("p i h q -> p i (h q)")
                    nc.scalar.activation(
                        ptf[:, :, relq:], qkf[:, :, relq:],
                        mybir.ActivationFunctionType.Exp, scale=SCALE,
                    )
                    st["act"] += 2 * (QC - relq) * ACT_NS + EXP_OVH
                    if rel >= 0:
                        nc.vector.tensor_mul(
                            ptf[:, :, relq:relq + P],
                            ptf[:, :, relq:relq + P],
                            mask_sb[:, rel, relq:relq + P][:, None, :]
                            .to_broadcast((P, 2, P)),
                        )
                        st["dve"] += 2 * P * 0.5 * DVE_NS + 121
                    pend.append((kt, ptf))
                    if len(pend) > LAG:
                        emit_pv(*pend.popleft())
                    pump()
                while pend:
                    emit_pv(*pend.popleft())
                    pump()

        # ---- epilogue: drain remaining fillers (c_proj tail on ACT) ----
        n_left = len(fillers)
        for i in range(n_left):
            it = fillers.popleft()
            if it[1] == "c" and i >= n_left - 2:
                emit_cproj(*it[2], on_act=True)
            else:
                emit_item(it)

    nc.compile()
    return nc


def make_in_maps(x, Wq, bq, Wk, bk, Wv, bv, T=T_FULL):
    """Host-side sharding + layout prep. Returns per-core input dicts."""
    bf = ml_dtypes.bfloat16
    x = np.asarray(x, dtype=np.float32)
    n_dt = CH // P

    # causal masks for the 4 diagonal-relative offsets
    k_idx = np.arange(P)[:, None]
    q_idx = np.arange(QC)[None, :]
    masks = np.concatenate(
        [(r * P + k_idx <= q_idx) for r in range(4)], axis=1
    ).astype(bf)  # [128, 4*512]

    # head-interleave permutation for Wv columns: new col j*HPC+h = old h*HD+j
    j = np.arange(HD)[:, None]
    h = np.arange(HPC)[None, :]
    perm = (h * HD + j).reshape(-1)

    wqT = np.ascontiguousarray(Wq.T).astype(bf)  # [cin, dout]
    wkT = np.ascontiguousarray(Wk.T).astype(bf)
    wvT = np.ascontiguousarray(Wv.T).astype(bf)

    in_maps = []
    for core in range(N_CORES):
        b = core // 2
        hh = core % 2
        cs = slice(hh * CH, (hh + 1) * CH)
        xtb = np.ascontiguousarray(x[b, :T].T).astype(bf)  # [N_EMBD, T]
        xt0 = np.ascontiguousarray(xtb[:, :QC])
        xtr = np.ascontiguousarray(xtb[:, QC:])

        bq_arr = np.asarray(bq[cs], np.float32).reshape(n_dt, P).T.copy()
        bk_arr = np.asarray(bk[cs], np.float32).reshape(n_dt, P).T.copy()
        bv_half = np.asarray(bv[cs], np.float32)
        vb1 = np.concatenate([bv_half[perm], np.ones(HPC, np.float32)])
        vb1 = np.broadcast_to(vb1, (P, CH + HPC)).copy()

        im = {
            "xt0": xt0,
            "wqkv": np.ascontiguousarray(np.concatenate(
                [wqT[:, cs], wkT[:, cs], wvT[:, cs][:, perm]], axis=1)),
            "wc": None,  # filled by caller (needs Wc)
            "bq": bq_arr,
            "bk": bk_arr,
            "vb1": vb1,
            "masks": masks,
        }
        if T > QC:
            im["xtr"] = xtr
        in_maps.append(im)
    return in_maps


_NC_CACHE = {}


def kernel(x, Wq, bq, Wk, bk, Wv, bv, Wc, bc):
    x = np.asarray(x, dtype=np.float32)
    T = x.shape[1]
    key = T
    if key not in _NC_CACHE:
        _NC_CACHE[key] = build_nc(T=T)
    nc = _NC_CACHE[key]

    in_maps = make_in_maps(x, Wq, bq, Wk, bk, Wv, bv, T=T)
    wcT = np.ascontiguousarray(np.asarray(Wc, np.float32).T).astype(
        ml_dtypes.bfloat16)  # [cin, cout]
    for core in range(N_CORES):
        hh = core % 2
        in_maps[core]["wc"] = np.ascontiguousarray(wcT[hh * CH:(hh + 1) * CH, :])

    res = run_bass_kernel_spmd(nc, in_maps, core_ids=list(range(N_CORES)))

    bc = np.asarray(bc, np.float32)
    out = np.empty((B, T, N_EMBD), np.float32)
    for b in range(B):
        out[b] = res.results[2 * b]["out"] + res.results[2 * b + 1]["out"] + bc
    return out
